# revision 40
# baseline (speedup 1.0000x reference)
"""4-bit quant linear (dense_mlp) on 8 TRN2 NeuronCores — v4.

out[m,o] = sum_i x[m,i] * (scales[o]*q[i,o] - zeros[o]) + bias[o]

Per core (2D shard: tokens 4-way x outfeatures 2-way):

  Weights: fp8e4m3 bit patterns 0..15 represent exactly n * 2^-9
  (subnormals + first normal octave are linear in the bit pattern; the
  PE multiplies them exactly — HW-validated).  qweight is byte-permuted
  on the host so two DVE uint16 bit-ops per row-chunk (AND 0x0F0F /
  SHR4+AND, 2x mode) yield all nibble planes CONTIGUOUS per plane.
  The 2^9 and the scales multiply fold into the fp32 epilogue; bias and
  the zero-point term ride a K=2 bf16 affine matmul against
  [1; rowsum(x-hat)] inside PSUM.

  x: measured HW rates are bf16 matmul ~220ns and fp8 DoubleRow ~215ns
  per [128,512] instruction (DoubleRow covers 2 k-planes = 2x bf16
  k-throughput; the 4x the cost model claims is wrong on HW).  So the
  fast path is plain fp8 on a fraction of k-planes bounded by the error
  budget, bf16 for the rest, both against the same fp8 weight tiles:
    - r-chunks 0..n_f8-1: ACT converts fp32 -> fp8 directly
      (plane-major permuted, accum_out = rowsum of the CONVERTED
      values, keeping the zero-point term consistent), fp8 PE
      transposes (PSUM element-step 2), one DVE strided copy per batch
      drains to packed lhsT pairs for DoubleRow.
    - remaining r-chunks: ACT converts fp32 -> bf16, bf16 PE
      transposes, DVE copy drains, plain bf16 matmuls.
  No DRAM staging, no X-bar transposes.
"""

import sys

if "/opt/trn_rl_repo" not in sys.path:
    sys.path.insert(0, "/opt/trn_rl_repo")

import numpy as np

import concourse.bass as bass
import concourse.tile as tile
from concourse import bacc, mybir
from concourse.masks import make_identity

B, S, IN, OUT = 4, 2048, 4096, 4096
PACK = 8
M_TOT = B * S
M_SPLIT, O_SPLIT = 4, 2
M_SH, O_SH = M_TOT // M_SPLIT, OUT // O_SPLIT
N_CORES = 8

P = 128
NB = 512  # o-block (one PSUM bank of fp32)
XC = 1024  # x chunk (128 qweight rows * 8 nibbles)
TB = 8  # k-planes per transpose batch (one PSUM bank)
BTG = 4  # token tiles per X-bar staging group

FP32 = mybir.dt.float32
BF16 = mybir.dt.bfloat16
FP8 = mybir.dt.float8e4
INT32 = mybir.dt.int32
U16 = mybir.dt.uint16
Alu = mybir.AluOpType
ACT_COPY = mybir.ActivationFunctionType.Copy
DR = mybir.MatmulPerfMode.DoubleRow


def build_kernel(
    m_sh=M_SH,
    o_sh=O_SH,
    in_dim=IN,
    bench_iters=1,
    kt8=12,  # k-planes (of 32) done in plain fp8 (DoubleRow pairs)
    bench_variant="full",  # full | mmonly | mmonly_noaffine | xpipe
):
    n_kt = in_dim // P  # 32 k-planes
    n_r = in_dim // XC  # 4 qweight row-chunks
    n_bt = m_sh // P  # 16 token tiles
    n_ob = o_sh // NB  # 4 o-blocks

    nc = bacc.Bacc(
        "TRN2", target_bir_lowering=False, debug=False, enable_asserts=False
    )
    x_d = nc.dram_tensor("x", [m_sh, in_dim], FP32, kind="ExternalInput").ap()
    qw_d = nc.dram_tensor(
        "qweight", [in_dim // PACK, o_sh], INT32, kind="ExternalInput"
    ).ap()
    scales2_d = nc.dram_tensor("scales2", [1, o_sh], FP32, kind="ExternalInput").ap()
    biasnz_d = nc.dram_tensor("biasnz", [2, o_sh], BF16, kind="ExternalInput").ap()
    out_d = nc.dram_tensor("out", [m_sh, o_sh], BF16, kind="ExternalOutput").ap()

    def bcast_ap(src, parts=P):
        return bass.AP(
            tensor=src.tensor, offset=src.offset, ap=[[0, parts]] + src.ap[1:]
        )

    with tile.TileContext(nc) as tc:
        with (
            tc.tile_pool(name="consts", bufs=1) as consts,
            tc.tile_pool(name="wpool", bufs=1) as wpool,
            tc.tile_pool(name="wdbp", bufs=2) as wdbp,
            tc.tile_pool(name="qwp", bufs=2) as qwp,
            tc.tile_pool(name="xp", bufs=3) as xp,
            tc.tile_pool(name="yp", bufs=2) as yp,
            tc.tile_pool(name="xfp", bufs=2) as xfp,
            tc.tile_pool(name="xf8p", bufs=2) as xf8p,
            tc.tile_pool(name="xbp", bufs=2) as xbp,
            tc.tile_pool(name="rsp", bufs=12) as rsp,
            tc.tile_pool(name="outp", bufs=4) as outp,
            tc.tile_pool(name="pst8", bufs=1, space="PSUM") as pst8,
            tc.tile_pool(name="pstb", bufs=2, space="PSUM") as pstb,
            tc.tile_pool(name="psr", bufs=1, space="PSUM") as psr,
            tc.tile_pool(name="psm", bufs=4, space="PSUM") as psm,
        ):
            # ---- constants ----
            identity = consts.tile([P, P], BF16)
            make_identity(nc, identity)
            identity8 = consts.tile([P, P], FP8)
            make_identity(nc, identity8)
            scales2_b = consts.tile([P, o_sh], FP32)
            nc.gpsimd.dma_start(out=scales2_b, in_=bcast_ap(scales2_d))
            biasnz = consts.tile([2, o_sh], BF16)
            nc.gpsimd.dma_start(out=biasnz, in_=biasnz_d)
            dummy = consts.tile([P, 64], FP32)

            # packed nibble-plane weights, one tile per r-chunk:
            # [p, s, t*o_sh + o] = plane (8r + 2t + s) at o-col o
            # (value n * 2^-9 as fp8; host byte-permute makes planes
            # contiguous).  r0/r1 rotate through a double-buffered pool
            # so their re-unpack at a For_i iteration boundary overlaps
            # the previous iteration's tail matmuls; r2/r3 persist.
            w_sb = [None, None] + [
                wpool.tile([P, 2, 4 * o_sh], FP8, name=f"w{r}")
                for r in range(2, n_r)
            ]

            cfg = dict(
                n_kt=n_kt, n_r=n_r, n_bt=n_bt, n_ob=n_ob, o_sh=o_sh,
                kt8=kt8, variant=bench_variant,
            )
            pools = dict(
                wdbp=wdbp,
                qwp=qwp, xp=xp, yp=yp, xfp=xfp, xf8p=xf8p, xbp=xbp,
                rsp=rsp, outp=outp, pst8=pst8, pstb=pstb, psr=psr, psm=psm,
            )
            tens = dict(
                identity=identity, identity8=identity8,
                scales2_b=scales2_b, biasnz=biasnz, dummy=dummy,
                x_d=x_d, qw_d=qw_d, out_d=out_d,
            )
            if bench_iters > 1:
                with tc.For_i(0, bench_iters, 1):
                    _pass_body(nc, pools, cfg, tens, w_sb)
            else:
                _pass_body(nc, pools, cfg, tens, w_sb)
    nc.compile()
    return nc


def _pass_body(nc, pools, cfg, tens, w_sb):
    qwp, xp, yp = pools["qwp"], pools["xp"], pools["yp"]
    xfp, xf8p, xbp = pools["xfp"], pools["xf8p"], pools["xbp"]
    rsp, outp = pools["rsp"], pools["outp"]
    pst8, pstb, psr, psm = (
        pools["pst8"], pools["pstb"], pools["psr"], pools["psm"],
    )
    n_kt, n_r, n_bt, n_ob = cfg["n_kt"], cfg["n_r"], cfg["n_bt"], cfg["n_ob"]
    o_sh, kt8 = cfg["o_sh"], cfg["kt8"]
    variant = cfg.get("variant", "full")
    identity, identity8 = tens["identity"], tens["identity8"]
    scales2_b, biasnz, dummy = tens["scales2_b"], tens["biasnz"], tens["dummy"]
    x_d, qw_d, out_d = tens["x_d"], tens["qw_d"], tens["out_d"]
    assert kt8 % 4 == 0
    n_bfp = n_kt - kt8  # bf16 k-planes

    if variant.startswith("mmonly"):
        _mmonly_body(nc, pools, cfg, tens, w_sb)
        return

    # ---- weight unpack: 2 DVE u16 ops per plane-pair chunk.  The
    # host byte-permute lays qweight bytes [t, o] (plane-pair-major),
    # so int32 chunk h of an r-chunk row-block holds exactly pair t=h.
    w_sb = list(w_sb)
    for r in range(2):
        w_sb[r] = pools["wdbp"].tile([P, 2, 4 * o_sh], FP8, name=f"w{r}")
    QWC = o_sh // 4  # int32 cols per plane-pair chunk
    for r in range(n_r):
        for h in range(4):
            qw_t = qwp.tile([P, QWC], INT32, name="qw_t")
            nc.gpsimd.dma_start(
                out=qw_t,
                in_=qw_d[r * P : (r + 1) * P, h * QWC : (h + 1) * QWC],
            )
            qw_u16 = qw_t[:, :].bitcast(U16)
            nc.vector.tensor_scalar(
                w_sb[r][:, 0, h * o_sh : (h + 1) * o_sh].bitcast(U16),
                qw_u16, 0x0F0F, None,
                op0=Alu.bitwise_and,
            )
            nc.vector.tensor_scalar(
                w_sb[r][:, 1, h * o_sh : (h + 1) * o_sh].bitcast(U16),
                qw_u16, 4, 0x0F0F,
                op0=Alu.logical_shift_right, op1=Alu.bitwise_and,
            )

    def w_plane(kp, osl):
        # single plane kp as [P, NB] contiguous fp8
        r, k = divmod(kp, PACK)
        t, s = divmod(k, 2)
        return w_sb[r][:, s, t * o_sh : (t + 1) * o_sh][:, osl]

    def w_pair(t, osl):
        # planes (2t, 2t+1) as [P, 2, NB] for DoubleRow
        r, tt = divmod(t, PACK // 2)
        return w_sb[r][:, :, tt * o_sh : (tt + 1) * o_sh][:, :, osl]

    lhs2_l = [None] * n_bt
    xf_l = [None] * n_bt
    xb_l = [None] * n_bt

    def stage_x(bt):
        # x load + plane-major permuted converts (+rowsums of the
        # converted values) + affine lhsT prep.  Engines: DMA, ACT,
        # (tiny PE/DVE).
        bsl = slice(bt * P, (bt + 1) * P)
        xf8_row = xf8p.tile([P, kt8 * P], FP8, name="xf8r") if kt8 else None
        y_t = yp.tile([P, n_bfp * P], BF16, name="y_t") if n_bfp else None
        rs_part = rsp.tile([P, n_r + 1], FP32, name="rs_part", bufs=4)
        nc.gpsimd.memset(rs_part[:, n_r : n_r + 1], 0.0)
        for r in range(n_r):
            x_t = xp.tile([P, XC], FP32, name="x_t")
            nc.sync.dma_start(out=x_t, in_=x_d[bsl, r * XC : (r + 1) * XC])
            x_r = x_t.rearrange("p (j e) -> p e j", e=PACK)
            # planes [8r, 8r+8): first nf8 in plain fp8, rest bf16.
            # fp8 accum_out would sum PRE-conversion values; the
            # zero-point term needs the rowsum of the CONVERTED fp8
            # values — accumulated in a second pass over xf8_row below.
            nf8 = max(0, min(PACK, kt8 - r * PACK))
            if nf8:
                dst = xf8_row[:, r * PACK * P : (r * PACK + nf8) * P]
                nc.scalar.activation(
                    dst.rearrange("p (e j) -> p e j", e=nf8),
                    x_r[:, 0:nf8, :], ACT_COPY, scale=1.0,
                )
            if nf8 < PACK:
                jb = r * PACK + nf8 - kt8  # local bf16 plane index
                dst = y_t[:, jb * P : (jb + PACK - nf8) * P]
                nc.scalar.activation(
                    dst.rearrange("p (e j) -> p e j", e=PACK - nf8),
                    x_r[:, nf8:PACK, :], ACT_COPY, scale=1.0,
                    accum_out=rs_part[:, r : r + 1],
                )
            else:
                nc.gpsimd.memset(rs_part[:, r : r + 1], 0.0)
        if kt8:
            scr = rsp.tile([P, kt8 * P], FP8, name="scr", bufs=2)
            nc.scalar.activation(
                scr, xf8_row, ACT_COPY,
                scale=1.0, accum_out=rs_part[:, n_r : n_r + 1],
            )
        rs_t = rsp.tile([P, 1], FP32, name="rs")
        nc.scalar.activation(
            dummy[:, : n_r + 1], rs_part, ACT_COPY, scale=1.0,
            accum_out=rs_t,
        )
        # rowsum -> [2,128] bf16 affine lhsT via PE transpose
        rs2 = rsp.tile([P, 2], BF16, name="rs2", bufs=4)
        nc.gpsimd.memset(rs2[:, 0:1], 1.0)
        nc.vector.tensor_copy(out=rs2[:, 1:2], in_=rs_t)
        ps_r = psr.tile([2, P], BF16, name="ps_r")
        nc.tensor.transpose(ps_r, rs2, identity)
        lhs2 = rsp.tile([2, P], BF16, name="lhs2")
        nc.vector.tensor_copy(out=lhs2, in_=ps_r)
        lhs2_l[bt] = lhs2
        return xf8_row, y_t

    def stage_t(bt, xf8_row, y_t):
        # PE transposes + DVE drains -> lhsT tiles for bt.  Emitted
        # BEFORE stage_mm(bt-1) so the drains overlap the previous
        # token tile's matmuls instead of serializing after them.
        xf_t = xfp.tile([P, kt8 * P], FP8, name="xf") if kt8 else None
        if variant == "notrans" and kt8:
            nc.gpsimd.memset(xf_t[:, :].bitcast(U16), 0)
        k0 = 0
        while variant != "notrans" and k0 < kt8:
            nb = min(TB, kt8 - k0)
            ps8 = pst8.tile([P, nb * P * 2], FP8, name="ps8")
            ps8_r = ps8.rearrange("p (k m two) -> p k m two", k=nb, two=2)
            for k in range(nb):
                kp = k0 + k
                nc.tensor.transpose(
                    ps8_r[:, k, :, 0:1],
                    xf8_row[:, kp * P : (kp + 1) * P],
                    identity8,
                )
            nc.vector.tensor_copy(
                out=xf_t[:, k0 * P : (k0 + nb) * P].rearrange(
                    "p (k m) -> p k m", k=nb
                ),
                in_=ps8_r[:, :, :, 0],
            )
            k0 += nb
        # ---- bf16 transposes + drains ----
        xb_t = xbp.tile([P, n_bfp * P], BF16, name="xb")
        if variant == "notrans":
            nc.gpsimd.memset(xb_t, 0.5)
        else:
            k0 = 0
            while k0 < n_bfp:
                nb = min(TB, n_bfp - k0)
                psb = pstb.tile([P, nb * P], BF16, name="psb")
                for k in range(nb):
                    kp = k0 + k
                    nc.tensor.transpose(
                        psb[:, k * P : (k + 1) * P],
                        y_t[:, kp * P : (kp + 1) * P],
                        identity,
                    )
                nc.vector.tensor_copy(
                    out=xb_t[:, k0 * P : (k0 + nb) * P], in_=psb
                )
                k0 += nb
        xf_l[bt] = xf_t
        xb_l[bt] = xb_t

        if variant == "xpipe":
            o_t = outp.tile([P, NB], BF16, name="o_t")
            nc.vector.tensor_copy(out=o_t[:, 0:P], in_=xf_t[:, 0:P])
            if bt == n_bt - 1:
                nc.sync.dma_start(
                    out=out_d[bt * P : (bt + 1) * P, 0:NB], in_=o_t
                )

    def stage_mm(bt):
        # matmul blocks: all DR pairs first across the o-blocks, then
        # all bf16 planes — minimizes DR<->bf16 perf-mode transitions
        # on the PE (measured ~300ns per switch)
        bsl = slice(bt * P, (bt + 1) * P)
        xf_t, xb_t, lhs2 = xf_l[bt], xb_l[bt], lhs2_l[bt]
        for oh in range(0, n_ob, 4):
            obs = range(oh, min(oh + 4, n_ob))
            ps_l = {ob: psm.tile([P, NB], FP32, name="ps") for ob in obs}
            for ob in obs:
                osl = slice(ob * NB, (ob + 1) * NB)
                for t in range(kt8 // 2):
                    nc.tensor.matmul(
                        ps_l[ob],
                        lhsT=xf_t[:, 2 * t * P : (2 * t + 2) * P].rearrange(
                            "p (s m) -> p s m", s=2
                        ),
                        rhs=w_pair(t, osl),
                        start=(t == 0), stop=False, perf_mode=DR,
                    )
            for ob in obs:
                osl = slice(ob * NB, (ob + 1) * NB)
                for j in range(n_bfp):
                    kp = kt8 + j
                    nc.tensor.matmul(
                        ps_l[ob],
                        lhsT=xb_t[:, j * P : (j + 1) * P],
                        rhs=w_plane(kp, osl),
                        start=(kt8 == 0 and j == 0), stop=False,
                    )
                # bias + rowsum*(-zeros) term inside PSUM (2^-9-scaled)
                nc.tensor.matmul(
                    ps_l[ob], lhsT=lhs2, rhs=biasnz[:, osl],
                    start=False, stop=True,
                )
                o_t = outp.tile([P, NB], BF16, name="o_t")
                nc.vector.tensor_tensor(
                    o_t, ps_l[ob], scales2_b[:, osl], op=Alu.mult
                )
                nc.sync.dma_start(out=out_d[bsl, osl], in_=o_t)

    # software pipeline: transposes for bt+1 land ahead of bt's
    # matmuls so their drains overlap the matmul stream.
    for bt in range(n_bt):
        xf8_row, y_t = stage_x(bt)
        stage_t(bt, xf8_row, y_t)
        if variant != "xpipe" and bt >= 1:
            stage_mm(bt - 1)
    if variant != "xpipe":
        stage_mm(n_bt - 1)


def _mmonly_body(nc, pools, cfg, tens, w_sb):
    xfp, xbp, rsp, outp, psm = (
        pools["xfp"], pools["xbp"], pools["rsp"], pools["outp"], pools["psm"],
    )
    n_bt, n_ob, o_sh, kt8 = (
        cfg["n_bt"], cfg["n_ob"], cfg["o_sh"], cfg["kt8"],
    )
    n_r = cfg["n_r"]
    scales2_b, biasnz = tens["scales2_b"], tens["biasnz"]
    out_d = tens["out_d"]
    n_bfp = cfg["n_kt"] - kt8

    w_sb = list(w_sb)
    for r in range(2):
        w_sb[r] = pools["wdbp"].tile([P, 2, 4 * o_sh], FP8, name=f"w{r}")
    for r in range(n_r):
        nc.gpsimd.memset(w_sb[r][:, :, :].bitcast(U16), 0)

    def w_plane(kp, osl):
        r, k = divmod(kp, PACK)
        t, s = divmod(k, 2)
        return w_sb[r][:, s, t * o_sh : (t + 1) * o_sh][:, osl]

    def w_pair(t, osl):
        r, tt = divmod(t, PACK // 2)
        return w_sb[r][:, :, tt * o_sh : (tt + 1) * o_sh][:, :, osl]

    xf_t = xfp.tile([P, kt8 * P], FP8, name="xf")
    nc.gpsimd.memset(xf_t[:, :].bitcast(U16), 0)
    xb_t = xbp.tile([P, n_bfp * P], BF16, name="xb")
    nc.gpsimd.memset(xb_t, 0.5)
    lhs2 = rsp.tile([2, P], BF16, name="lhs2")
    nc.gpsimd.memset(lhs2, 1.0)

    for bt in range(n_bt):
        bsl = slice(bt * P, (bt + 1) * P)
        for oh in range(0, n_ob, 4):
          obs = range(oh, min(oh + 4, n_ob))
          ps_l = {ob: psm.tile([P, NB], FP32, name="ps") for ob in obs}
          for ob in obs:
            osl = slice(ob * NB, (ob + 1) * NB)
            for t in range(kt8 // 2):
                nc.tensor.matmul(
                    ps_l[ob],
                    lhsT=xf_t[:, 2 * t * P : (2 * t + 2) * P].rearrange(
                        "p (s m) -> p s m", s=2
                    ),
                    rhs=w_pair(t, osl),
                    start=(t == 0), stop=False, perf_mode=DR,
                )
          for ob in obs:
            osl = slice(ob * NB, (ob + 1) * NB)
            for j in range(n_bfp):
                kp = kt8 + j
                nc.tensor.matmul(
                    ps_l[ob],
                    lhsT=xb_t[:, j * P : (j + 1) * P],
                    rhs=w_plane(kp, osl),
                    start=(kt8 == 0 and j == 0), stop=False,
                )
            if cfg.get("variant") != "mmonly_noaffine":
                nc.tensor.matmul(
                    ps_l[ob], lhsT=lhs2, rhs=biasnz[:, osl],
                    start=False, stop=True,
                )
            else:
                nc.tensor.matmul(
                    ps_l[ob],
                    lhsT=xb_t[:, 0:P],
                    rhs=w_plane(kt8, osl),
                    start=False, stop=True,
                )
            o_t = outp.tile([P, NB], BF16, name="o_t")
            nc.vector.tensor_tensor(
                o_t, ps_l[ob], scales2_b[:, osl], op=Alu.mult
            )
            nc.sync.dma_start(out=out_d[bsl, osl], in_=o_t)


_nc_full = None
_nc_cfg = None


def _shard_inputs(x, qweight, scales, zeros, bias):
    import ml_dtypes

    x_flat = np.ascontiguousarray(x.reshape(M_TOT, IN), dtype=np.float32)
    scales_f = np.asarray(scales, dtype=np.float32).reshape(OUT)
    zeros_f = np.asarray(zeros, dtype=np.float32).reshape(OUT)
    bias_f = np.asarray(bias, dtype=np.float32).reshape(OUT)
    scales2_full = scales_f * 512.0
    biasnz_full = np.stack(
        [bias_f / scales2_full, -zeros_f / scales2_full]
    ).astype(ml_dtypes.bfloat16)
    in_maps = []
    for c in range(N_CORES):
        mb_, ob = divmod(c, O_SPLIT)
        osl = slice(ob * O_SH, (ob + 1) * O_SH)
        qw_sh = np.ascontiguousarray(qweight[:, osl])
        # byte-permute: word bytes beta=0..3 (nibble pairs) become
        # plane-pair-major: bytes [rows, o, beta] -> [rows, beta, o]
        qw_perm = (
            np.ascontiguousarray(
                qw_sh.view(np.uint8)
                .reshape(IN // PACK, O_SH, 4)
                .transpose(0, 2, 1)
            )
            .reshape(IN // PACK, 4 * O_SH)
            .view(np.int32)
        )
        in_maps.append(
            {
                "x": np.ascontiguousarray(x_flat[mb_ * M_SH : (mb_ + 1) * M_SH]),
                "qweight": qw_perm,
                "scales2": np.ascontiguousarray(scales2_full[osl][None, :]),
                "biasnz": np.ascontiguousarray(biasnz_full[:, osl]),
            }
        )
    return in_maps


def kernel(x, qweight, scales, zeros, bias):
    global _nc_full
    from concourse import bass_utils

    if _nc_full is None:
        _nc_full = build_kernel()
    in_maps = _shard_inputs(
        np.asarray(x),
        np.asarray(qweight),
        np.asarray(scales),
        np.asarray(zeros),
        np.asarray(bias),
    )
    res = bass_utils.run_bass_kernel_spmd(
        _nc_full, in_maps, core_ids=list(range(N_CORES))
    )
    out = np.empty((M_TOT, OUT), np.float32)
    for c in range(N_CORES):
        mb_, ob = divmod(c, O_SPLIT)
        out[mb_ * M_SH : (mb_ + 1) * M_SH, ob * O_SH : (ob + 1) * O_SH] = res.results[
            c
        ]["out"].astype(np.float32)
    return out.reshape(B, S, OUT)


# revision 41
# speedup vs baseline: 1.0443x; 1.0443x over previous
"""4-bit quant linear (dense_mlp) on 8 TRN2 NeuronCores — v4.

out[m,o] = sum_i x[m,i] * (scales[o]*q[i,o] - zeros[o]) + bias[o]

Per core (2D shard: tokens 4-way x outfeatures 2-way):

  Weights: fp8e4m3 bit patterns 0..15 represent exactly n * 2^-9
  (subnormals + first normal octave are linear in the bit pattern; the
  PE multiplies them exactly — HW-validated).  qweight is byte-permuted
  on the host so two DVE uint16 bit-ops per row-chunk (AND 0x0F0F /
  SHR4+AND, 2x mode) yield all nibble planes CONTIGUOUS per plane.
  The 2^9 and the scales multiply fold into the fp32 epilogue; bias and
  the zero-point term ride a K=2 bf16 affine matmul against
  [1; rowsum(x-hat)] inside PSUM.

  x: measured HW rates are bf16 matmul ~220ns and fp8 DoubleRow ~215ns
  per [128,512] instruction (DoubleRow covers 2 k-planes = 2x bf16
  k-throughput; the 4x the cost model claims is wrong on HW).  So the
  fast path is plain fp8 on a fraction of k-planes bounded by the error
  budget, bf16 for the rest, both against the same fp8 weight tiles:
    - r-chunks 0..n_f8-1: ACT converts fp32 -> fp8 directly
      (plane-major permuted, accum_out = rowsum of the CONVERTED
      values, keeping the zero-point term consistent), fp8 PE
      transposes (PSUM element-step 2), one DVE strided copy per batch
      drains to packed lhsT pairs for DoubleRow.
    - remaining r-chunks: ACT converts fp32 -> bf16, bf16 PE
      transposes, DVE copy drains, plain bf16 matmuls.
  No DRAM staging, no X-bar transposes.
"""

import sys

if "/opt/trn_rl_repo" not in sys.path:
    sys.path.insert(0, "/opt/trn_rl_repo")

import numpy as np

import concourse.bass as bass
import concourse.tile as tile
from concourse import bacc, mybir
from concourse.masks import make_identity

B, S, IN, OUT = 4, 2048, 4096, 4096
PACK = 8
M_TOT = B * S
M_SPLIT, O_SPLIT = 4, 2
M_SH, O_SH = M_TOT // M_SPLIT, OUT // O_SPLIT
N_CORES = 8

P = 128
NB = 512  # o-block (one PSUM bank of fp32)
XC = 1024  # x chunk (128 qweight rows * 8 nibbles)
TB = 8  # k-planes per transpose batch (one PSUM bank)
BTG = 4  # token tiles per X-bar staging group

FP32 = mybir.dt.float32
BF16 = mybir.dt.bfloat16
FP8 = mybir.dt.float8e4
INT32 = mybir.dt.int32
U16 = mybir.dt.uint16
Alu = mybir.AluOpType
ACT_COPY = mybir.ActivationFunctionType.Copy
DR = mybir.MatmulPerfMode.DoubleRow


def build_kernel(
    m_sh=M_SH,
    o_sh=O_SH,
    in_dim=IN,
    bench_iters=1,
    kt8=12,  # k-planes (of 32) done in plain fp8 (DoubleRow pairs)
    bench_variant="full",  # full | mmonly | mmonly_noaffine | xpipe
):
    n_kt = in_dim // P  # 32 k-planes
    n_r = in_dim // XC  # 4 qweight row-chunks
    n_bt = m_sh // P  # 16 token tiles
    n_ob = o_sh // NB  # 4 o-blocks

    nc = bacc.Bacc(
        "TRN2", target_bir_lowering=False, debug=False, enable_asserts=False
    )
    x_d = nc.dram_tensor("x", [m_sh, in_dim], FP32, kind="ExternalInput").ap()
    qw_d = nc.dram_tensor(
        "qweight", [in_dim // PACK, o_sh], INT32, kind="ExternalInput"
    ).ap()
    scales2_d = nc.dram_tensor("scales2", [1, o_sh], FP32, kind="ExternalInput").ap()
    biasnz_d = nc.dram_tensor("biasnz", [2, o_sh], BF16, kind="ExternalInput").ap()
    out_d = nc.dram_tensor("out", [m_sh, o_sh], BF16, kind="ExternalOutput").ap()

    def bcast_ap(src, parts=P):
        return bass.AP(
            tensor=src.tensor, offset=src.offset, ap=[[0, parts]] + src.ap[1:]
        )

    with tile.TileContext(nc) as tc:
        with (
            tc.tile_pool(name="consts", bufs=1) as consts,
            tc.tile_pool(name="wpool", bufs=1) as wpool,
            tc.tile_pool(name="qwp", bufs=2) as qwp,
            tc.tile_pool(name="xp", bufs=3) as xp,
            tc.tile_pool(name="yp", bufs=2) as yp,
            tc.tile_pool(name="xfp", bufs=2) as xfp,
            tc.tile_pool(name="xf8p", bufs=2) as xf8p,
            tc.tile_pool(name="xbp", bufs=2) as xbp,
            tc.tile_pool(name="rsp", bufs=12) as rsp,
            tc.tile_pool(name="outp", bufs=4) as outp,
            tc.tile_pool(name="pst8", bufs=1, space="PSUM") as pst8,
            tc.tile_pool(name="pstb", bufs=2, space="PSUM") as pstb,
            tc.tile_pool(name="psr", bufs=1, space="PSUM") as psr,
            tc.tile_pool(name="psm", bufs=4, space="PSUM") as psm,
        ):
            # ---- constants ----
            identity = consts.tile([P, P], BF16)
            make_identity(nc, identity)
            identity8 = consts.tile([P, P], FP8)
            make_identity(nc, identity8)
            scales2_b = consts.tile([P, o_sh], FP32)
            nc.gpsimd.dma_start(out=scales2_b, in_=bcast_ap(scales2_d))
            biasnz = consts.tile([2, o_sh], BF16)
            nc.gpsimd.dma_start(out=biasnz, in_=biasnz_d)
            dummy = consts.tile([P, 64], FP32)

            # packed nibble-plane weights, one tile per r-chunk:
            # [p, s, t*o_sh + o] = plane (8r + 2t + s) at o-col o
            # (value n * 2^-9 as fp8; host byte-permute makes planes
            # contiguous).
            w_sb = [
                wpool.tile([P, 2, 4 * o_sh], FP8, name=f"w{r}")
                for r in range(n_r)
            ]

            cfg = dict(
                n_kt=n_kt, n_r=n_r, n_bt=n_bt, n_ob=n_ob, o_sh=o_sh,
                kt8=kt8, variant=bench_variant,
            )
            pools = dict(
                qwp=qwp, xp=xp, yp=yp, xfp=xfp, xf8p=xf8p, xbp=xbp,
                rsp=rsp, outp=outp, pst8=pst8, pstb=pstb, psr=psr, psm=psm,
            )
            tens = dict(
                identity=identity, identity8=identity8,
                scales2_b=scales2_b, biasnz=biasnz, dummy=dummy,
                x_d=x_d, qw_d=qw_d, out_d=out_d,
            )
            if bench_iters > 1:
                with tc.For_i(0, bench_iters, 1):
                    _pass_body(nc, pools, cfg, tens, w_sb)
            else:
                _pass_body(nc, pools, cfg, tens, w_sb)
    nc.compile()
    return nc


def _pass_body(nc, pools, cfg, tens, w_sb):
    qwp, xp, yp = pools["qwp"], pools["xp"], pools["yp"]
    xfp, xf8p, xbp = pools["xfp"], pools["xf8p"], pools["xbp"]
    rsp, outp = pools["rsp"], pools["outp"]
    pst8, pstb, psr, psm = (
        pools["pst8"], pools["pstb"], pools["psr"], pools["psm"],
    )
    n_kt, n_r, n_bt, n_ob = cfg["n_kt"], cfg["n_r"], cfg["n_bt"], cfg["n_ob"]
    o_sh, kt8 = cfg["o_sh"], cfg["kt8"]
    variant = cfg.get("variant", "full")
    identity, identity8 = tens["identity"], tens["identity8"]
    scales2_b, biasnz, dummy = tens["scales2_b"], tens["biasnz"], tens["dummy"]
    x_d, qw_d, out_d = tens["x_d"], tens["qw_d"], tens["out_d"]
    assert kt8 % 4 == 0
    n_bfp = n_kt - kt8  # bf16 k-planes

    if variant.startswith("mmonly"):
        _mmonly_body(nc, pools, cfg, tens, w_sb)
        return

    # ---- weight unpack: 2 DVE u16 ops per plane-pair chunk.  The
    # host byte-permute lays qweight bytes [t, o] (plane-pair-major),
    # so int32 chunk h of an r-chunk row-block holds exactly pair t=h.
    QWC = o_sh // 4  # int32 cols per plane-pair chunk
    for r in range(n_r):
        for h in range(4):
            qw_t = qwp.tile([P, QWC], INT32, name="qw_t")
            nc.gpsimd.dma_start(
                out=qw_t,
                in_=qw_d[r * P : (r + 1) * P, h * QWC : (h + 1) * QWC],
            )
            qw_u16 = qw_t[:, :].bitcast(U16)
            nc.vector.tensor_scalar(
                w_sb[r][:, 0, h * o_sh : (h + 1) * o_sh].bitcast(U16),
                qw_u16, 0x0F0F, None,
                op0=Alu.bitwise_and,
            )
            nc.vector.tensor_scalar(
                w_sb[r][:, 1, h * o_sh : (h + 1) * o_sh].bitcast(U16),
                qw_u16, 4, 0x0F0F,
                op0=Alu.logical_shift_right, op1=Alu.bitwise_and,
            )

    def w_plane(kp, osl):
        # single plane kp as [P, NB] contiguous fp8
        r, k = divmod(kp, PACK)
        t, s = divmod(k, 2)
        return w_sb[r][:, s, t * o_sh : (t + 1) * o_sh][:, osl]

    def w_pair(t, osl):
        # planes (2t, 2t+1) as [P, 2, NB] for DoubleRow
        r, tt = divmod(t, PACK // 2)
        return w_sb[r][:, :, tt * o_sh : (tt + 1) * o_sh][:, :, osl]

    lhs2_l = [None] * n_bt
    xf_l = [None] * n_bt
    xb_l = [None] * n_bt

    def stage_x(bt):
        # x load + plane-major permuted converts (+rowsums of the
        # converted values) + affine lhsT prep.  Engines: DMA, ACT,
        # (tiny PE/DVE).
        bsl = slice(bt * P, (bt + 1) * P)
        xf8_row = xf8p.tile([P, kt8 * P], FP8, name="xf8r") if kt8 else None
        y_t = yp.tile([P, n_bfp * P], BF16, name="y_t") if n_bfp else None
        rs_part = rsp.tile([P, n_r + 1], FP32, name="rs_part", bufs=4)
        nc.gpsimd.memset(rs_part[:, n_r : n_r + 1], 0.0)
        for r in range(n_r):
            x_t = xp.tile([P, XC], FP32, name="x_t")
            nc.sync.dma_start(out=x_t, in_=x_d[bsl, r * XC : (r + 1) * XC])
            x_r = x_t.rearrange("p (j e) -> p e j", e=PACK)
            # planes [8r, 8r+8): first nf8 in plain fp8, rest bf16.
            # fp8 accum_out would sum PRE-conversion values; the
            # zero-point term needs the rowsum of the CONVERTED fp8
            # values — accumulated in a second pass over xf8_row below.
            nf8 = max(0, min(PACK, kt8 - r * PACK))
            if nf8:
                dst = xf8_row[:, r * PACK * P : (r * PACK + nf8) * P]
                nc.scalar.activation(
                    dst.rearrange("p (e j) -> p e j", e=nf8),
                    x_r[:, 0:nf8, :], ACT_COPY, scale=1.0,
                )
            if nf8 < PACK:
                jb = r * PACK + nf8 - kt8  # local bf16 plane index
                dst = y_t[:, jb * P : (jb + PACK - nf8) * P]
                nc.scalar.activation(
                    dst.rearrange("p (e j) -> p e j", e=PACK - nf8),
                    x_r[:, nf8:PACK, :], ACT_COPY, scale=1.0,
                    accum_out=rs_part[:, r : r + 1],
                )
            else:
                nc.gpsimd.memset(rs_part[:, r : r + 1], 0.0)
        if kt8:
            scr = rsp.tile([P, kt8 * P], FP8, name="scr", bufs=2)
            nc.scalar.activation(
                scr, xf8_row, ACT_COPY,
                scale=1.0, accum_out=rs_part[:, n_r : n_r + 1],
            )
        rs_t = rsp.tile([P, 1], FP32, name="rs")
        nc.scalar.activation(
            dummy[:, : n_r + 1], rs_part, ACT_COPY, scale=1.0,
            accum_out=rs_t,
        )
        # rowsum -> [2,128] bf16 affine lhsT via PE transpose
        rs2 = rsp.tile([P, 2], BF16, name="rs2", bufs=4)
        nc.gpsimd.memset(rs2[:, 0:1], 1.0)
        nc.vector.tensor_copy(out=rs2[:, 1:2], in_=rs_t)
        ps_r = psr.tile([2, P], BF16, name="ps_r")
        nc.tensor.transpose(ps_r, rs2, identity)
        lhs2 = rsp.tile([2, P], BF16, name="lhs2")
        nc.vector.tensor_copy(out=lhs2, in_=ps_r)
        lhs2_l[bt] = lhs2
        return xf8_row, y_t

    def stage_t(bt, xf8_row, y_t):
        # PE transposes + DVE drains -> lhsT tiles for bt.  Emitted
        # BEFORE stage_mm(bt-1) so the drains overlap the previous
        # token tile's matmuls instead of serializing after them.
        xf_t = xfp.tile([P, kt8 * P], FP8, name="xf") if kt8 else None
        if variant == "notrans" and kt8:
            nc.gpsimd.memset(xf_t[:, :].bitcast(U16), 0)
        k0 = 0
        while variant != "notrans" and k0 < kt8:
            nb = min(TB, kt8 - k0)
            ps8 = pst8.tile([P, nb * P * 2], FP8, name="ps8")
            ps8_r = ps8.rearrange("p (k m two) -> p k m two", k=nb, two=2)
            for k in range(nb):
                kp = k0 + k
                nc.tensor.transpose(
                    ps8_r[:, k, :, 0:1],
                    xf8_row[:, kp * P : (kp + 1) * P],
                    identity8,
                )
            nc.vector.tensor_copy(
                out=xf_t[:, k0 * P : (k0 + nb) * P].rearrange(
                    "p (k m) -> p k m", k=nb
                ),
                in_=ps8_r[:, :, :, 0],
            )
            k0 += nb
        # ---- bf16 transposes + drains ----
        xb_t = xbp.tile([P, n_bfp * P], BF16, name="xb")
        if variant == "notrans":
            nc.gpsimd.memset(xb_t, 0.5)
        else:
            k0 = 0
            while k0 < n_bfp:
                nb = min(TB, n_bfp - k0)
                psb = pstb.tile([P, nb * P], BF16, name="psb")
                for k in range(nb):
                    kp = k0 + k
                    nc.tensor.transpose(
                        psb[:, k * P : (k + 1) * P],
                        y_t[:, kp * P : (kp + 1) * P],
                        identity,
                    )
                nc.vector.tensor_copy(
                    out=xb_t[:, k0 * P : (k0 + nb) * P], in_=psb
                )
                k0 += nb
        xf_l[bt] = xf_t
        xb_l[bt] = xb_t

        if variant == "xpipe":
            o_t = outp.tile([P, NB], BF16, name="o_t")
            nc.vector.tensor_copy(out=o_t[:, 0:P], in_=xf_t[:, 0:P])
            if bt == n_bt - 1:
                nc.sync.dma_start(
                    out=out_d[bt * P : (bt + 1) * P, 0:NB], in_=o_t
                )

    def stage_mm(bt):
        # matmul blocks: all DR pairs first across the o-blocks, then
        # all bf16 planes — minimizes DR<->bf16 perf-mode transitions
        # on the PE (measured ~300ns per switch)
        bsl = slice(bt * P, (bt + 1) * P)
        xf_t, xb_t, lhs2 = xf_l[bt], xb_l[bt], lhs2_l[bt]
        for oh in range(0, n_ob, 4):
            obs = range(oh, min(oh + 4, n_ob))
            ps_l = {ob: psm.tile([P, NB], FP32, name="ps") for ob in obs}
            for ob in obs:
                osl = slice(ob * NB, (ob + 1) * NB)
                for t in range(kt8 // 2):
                    nc.tensor.matmul(
                        ps_l[ob],
                        lhsT=xf_t[:, 2 * t * P : (2 * t + 2) * P].rearrange(
                            "p (s m) -> p s m", s=2
                        ),
                        rhs=w_pair(t, osl),
                        start=(t == 0), stop=False, perf_mode=DR,
                    )
            for ob in obs:
                osl = slice(ob * NB, (ob + 1) * NB)
                for j in range(n_bfp):
                    kp = kt8 + j
                    nc.tensor.matmul(
                        ps_l[ob],
                        lhsT=xb_t[:, j * P : (j + 1) * P],
                        rhs=w_plane(kp, osl),
                        start=(kt8 == 0 and j == 0), stop=False,
                    )
                # bias + rowsum*(-zeros) term inside PSUM (2^-9-scaled)
                nc.tensor.matmul(
                    ps_l[ob], lhsT=lhs2, rhs=biasnz[:, osl],
                    start=False, stop=True,
                )
                o_t = outp.tile([P, NB], BF16, name="o_t")
                nc.vector.tensor_tensor(
                    o_t, ps_l[ob], scales2_b[:, osl], op=Alu.mult
                )
                nc.sync.dma_start(out=out_d[bsl, osl], in_=o_t)

    # software pipeline: transposes for bt+1 land ahead of bt's
    # matmuls so their drains overlap the matmul stream.
    for bt in range(n_bt):
        xf8_row, y_t = stage_x(bt)
        stage_t(bt, xf8_row, y_t)
        if variant != "xpipe" and bt >= 1:
            stage_mm(bt - 1)
    if variant != "xpipe":
        stage_mm(n_bt - 1)


def _mmonly_body(nc, pools, cfg, tens, w_sb):
    xfp, xbp, rsp, outp, psm = (
        pools["xfp"], pools["xbp"], pools["rsp"], pools["outp"], pools["psm"],
    )
    n_bt, n_ob, o_sh, kt8 = (
        cfg["n_bt"], cfg["n_ob"], cfg["o_sh"], cfg["kt8"],
    )
    n_r = cfg["n_r"]
    scales2_b, biasnz = tens["scales2_b"], tens["biasnz"]
    out_d = tens["out_d"]
    n_bfp = cfg["n_kt"] - kt8

    for r in range(n_r):
        nc.gpsimd.memset(w_sb[r][:, :, :].bitcast(U16), 0)

    def w_plane(kp, osl):
        r, k = divmod(kp, PACK)
        t, s = divmod(k, 2)
        return w_sb[r][:, s, t * o_sh : (t + 1) * o_sh][:, osl]

    def w_pair(t, osl):
        r, tt = divmod(t, PACK // 2)
        return w_sb[r][:, :, tt * o_sh : (tt + 1) * o_sh][:, :, osl]

    xf_t = xfp.tile([P, kt8 * P], FP8, name="xf")
    nc.gpsimd.memset(xf_t[:, :].bitcast(U16), 0)
    xb_t = xbp.tile([P, n_bfp * P], BF16, name="xb")
    nc.gpsimd.memset(xb_t, 0.5)
    lhs2 = rsp.tile([2, P], BF16, name="lhs2")
    nc.gpsimd.memset(lhs2, 1.0)

    for bt in range(n_bt):
        bsl = slice(bt * P, (bt + 1) * P)
        for oh in range(0, n_ob, 4):
          obs = range(oh, min(oh + 4, n_ob))
          ps_l = {ob: psm.tile([P, NB], FP32, name="ps") for ob in obs}
          for ob in obs:
            osl = slice(ob * NB, (ob + 1) * NB)
            for t in range(kt8 // 2):
                nc.tensor.matmul(
                    ps_l[ob],
                    lhsT=xf_t[:, 2 * t * P : (2 * t + 2) * P].rearrange(
                        "p (s m) -> p s m", s=2
                    ),
                    rhs=w_pair(t, osl),
                    start=(t == 0), stop=False, perf_mode=DR,
                )
          for ob in obs:
            osl = slice(ob * NB, (ob + 1) * NB)
            for j in range(n_bfp):
                kp = kt8 + j
                nc.tensor.matmul(
                    ps_l[ob],
                    lhsT=xb_t[:, j * P : (j + 1) * P],
                    rhs=w_plane(kp, osl),
                    start=(kt8 == 0 and j == 0), stop=False,
                )
            if cfg.get("variant") != "mmonly_noaffine":
                nc.tensor.matmul(
                    ps_l[ob], lhsT=lhs2, rhs=biasnz[:, osl],
                    start=False, stop=True,
                )
            else:
                nc.tensor.matmul(
                    ps_l[ob],
                    lhsT=xb_t[:, 0:P],
                    rhs=w_plane(kt8, osl),
                    start=False, stop=True,
                )
            o_t = outp.tile([P, NB], BF16, name="o_t")
            nc.vector.tensor_tensor(
                o_t, ps_l[ob], scales2_b[:, osl], op=Alu.mult
            )
            nc.sync.dma_start(out=out_d[bsl, osl], in_=o_t)


_nc_full = None
_nc_cfg = None


def _shard_inputs(x, qweight, scales, zeros, bias):
    import ml_dtypes

    x_flat = np.ascontiguousarray(x.reshape(M_TOT, IN), dtype=np.float32)
    scales_f = np.asarray(scales, dtype=np.float32).reshape(OUT)
    zeros_f = np.asarray(zeros, dtype=np.float32).reshape(OUT)
    bias_f = np.asarray(bias, dtype=np.float32).reshape(OUT)
    scales2_full = scales_f * 512.0
    biasnz_full = np.stack(
        [bias_f / scales2_full, -zeros_f / scales2_full]
    ).astype(ml_dtypes.bfloat16)
    in_maps = []
    for c in range(N_CORES):
        mb_, ob = divmod(c, O_SPLIT)
        osl = slice(ob * O_SH, (ob + 1) * O_SH)
        qw_sh = np.ascontiguousarray(qweight[:, osl])
        # byte-permute: word bytes beta=0..3 (nibble pairs) become
        # plane-pair-major: bytes [rows, o, beta] -> [rows, beta, o]
        qw_perm = (
            np.ascontiguousarray(
                qw_sh.view(np.uint8)
                .reshape(IN // PACK, O_SH, 4)
                .transpose(0, 2, 1)
            )
            .reshape(IN // PACK, 4 * O_SH)
            .view(np.int32)
        )
        in_maps.append(
            {
                "x": np.ascontiguousarray(x_flat[mb_ * M_SH : (mb_ + 1) * M_SH]),
                "qweight": qw_perm,
                "scales2": np.ascontiguousarray(scales2_full[osl][None, :]),
                "biasnz": np.ascontiguousarray(biasnz_full[:, osl]),
            }
        )
    return in_maps


def kernel(x, qweight, scales, zeros, bias):
    global _nc_full
    from concourse import bass_utils

    if _nc_full is None:
        _nc_full = build_kernel()
    in_maps = _shard_inputs(
        np.asarray(x),
        np.asarray(qweight),
        np.asarray(scales),
        np.asarray(zeros),
        np.asarray(bias),
    )
    res = bass_utils.run_bass_kernel_spmd(
        _nc_full, in_maps, core_ids=list(range(N_CORES))
    )
    out = np.empty((M_TOT, OUT), np.float32)
    for c in range(N_CORES):
        mb_, ob = divmod(c, O_SPLIT)
        out[mb_ * M_SH : (mb_ + 1) * M_SH, ob * O_SH : (ob + 1) * O_SH] = res.results[
            c
        ]["out"].astype(np.float32)
    return out.reshape(B, S, OUT)
